# revision 13
# baseline (speedup 1.0000x reference)
"""CaptioningRNN (attention LSTM + vocab softmax loss) on 8 TRN2 NeuronCores.

Data-parallel over batch N=256 -> 32 samples/core. Weights replicated.
Matmuls bf16 (fp32 PSUM accumulate) except the attention-score and vocab
projections which run fp8e4m3 with DoubleRow perf mode (2 K-chunks per
MM).  The vocab GEMM is interleaved into the recurrence so the PE never
idles long enough for the HAM clock gate to re-throttle, and the target
score reduction runs per-step on the (otherwise idle) GpSimd engine.

Layouts (per core, B=32 samples, S=31 steps, H=1024, P16=16 spatial):
  - hT chunk order is permuted: position p holds h-dim chunk
    CHUNK_ORDER[p] = 4*(p%2) + p//2.  This lets the per-step h transpose
    run as 4x [64,128] PE transposes (each produces chunk pair {m, m+4}
    contiguously).  All h-contracted weights (Wh, Wattn, W_vocab, wtgt)
    are row-permuted on the host to match.
  - Gate GEMM: psum tiles (128,512) pack 4 units of 32 batch rows via PE
    column tiling (tile_position).  Emission is k-outer/unit-inner so the
    4 column groups stream concurrently.
  - c state lives in cc[64:128]; tanh(g) is written to cc[0:64] so the
    whole LSTM cell update runs as a few [64..128,512] DVE ops.
  - The per-step critical chain (scores -> softmax -> wT -> attn ->
    gates -> hT) is emitted under high_priority so background vocab MMs
    never delay it.
"""

import os
import numpy as np
import ml_dtypes

BF = ml_dtypes.bfloat16
F8 = ml_dtypes.float8_e4m3

N, T, V, W_DIM, H, D_IMG = 256, 32, 10000, 512, 1024, 1280
P16 = 16
NC = 8
B = N // NC          # 32 samples per core
S = T - 1            # 31 steps
ROWS = B * S         # 992 (t,n) rows per core, r = 32*t + n
VCH = 20             # vocab col chunks
VCOL = V // VCH      # 500
NEG = -1.0e5         # mask value (exp underflows to exactly 0)
NBLK = 8             # vocab row blocks of 128 rows (last one 96)
H_SCALE = 16.0       # h -> fp8 scale
W_SCALE = 32.0       # W_vocab -> fp8 scale
PRIO = 100000        # priority lift for the per-step critical chain
CHUNK_ORDER = [0, 4, 1, 5, 2, 6, 3, 7]   # pos -> h-dim chunk
POS = [0, 2, 4, 6, 1, 3, 5, 7]           # h-dim chunk -> pos

_cache = {}

last_exec_ns = None


def _build(has_b, has_bvocab):
    import concourse.mybir as mybir
    from concourse.bacc import Bacc
    from concourse.tile import TileContext
    import concourse.bass_isa as bass_isa

    F32 = mybir.dt.float32
    BF16 = mybir.dt.bfloat16
    FP8 = mybir.dt.float8e4
    DR = mybir.MatmulPerfMode.DoubleRow
    AF = mybir.ActivationFunctionType
    ALU = mybir.AluOpType
    AX = mybir.AxisListType

    nc = Bacc()

    d_f2t = nc.declare_dram_parameter("f2t", [1408, 512], BF16, isOutput=False)
    d_wproj = nc.declare_dram_parameter("wproj", [1408, 1024], BF16, isOutput=False)
    d_wattn = nc.declare_dram_parameter("wattn", [1024, 4096], BF16, isOutput=False)
    d_wh = nc.declare_dram_parameter("wh", [1024, 4096], BF16, isOutput=False)
    d_wx = nc.declare_dram_parameter("wx", [512, 4096], BF16, isOutput=False)
    d_xt = nc.declare_dram_parameter("xt", [512, ROWS], BF16, isOutput=False)
    d_wvoc8 = nc.declare_dram_parameter("wvoc8", [1024, V], FP8, isOutput=False)
    d_wtgt = nc.declare_dram_parameter("wtgt", [1024, ROWS], BF16, isOutput=False)
    d_maskm = nc.declare_dram_parameter("maskm", [128, NBLK], F32, isOutput=False)
    d_i128 = nc.declare_dram_parameter("i128", [128, 128], BF16, isOutput=False)
    d_m32 = nc.declare_dram_parameter("m32", [32, 512], BF16, isOutput=False)
    if has_b:
        d_bvec = nc.declare_dram_parameter("bvec", [1, 4096], BF16, isOutput=False)
    if has_bvocab:
        d_bvoc = nc.declare_dram_parameter("bvoc", [1, V], F32, isOutput=False)
        d_btgt = nc.declare_dram_parameter("btgt", [1, ROWS], F32, isOutput=False)
    d_loss = nc.declare_dram_parameter("loss", [1, 1], F32, isOutput=True)

    units = [(0, 0), (0, 1), (1, 0), (1, 1),
             (2, 0), (2, 1), (3, 0), (3, 1)]

    with TileContext(nc) as tc:
        with (
            tc.tile_pool(name="ppa", bufs=1) as ppa,
            tc.tile_pool(name="ppb", bufs=1) as ppb,
        ):
            # ---- persistent tiles ----
            at_t = ppa.tile([128, 8, 512], BF16, tag="at")        # A2T, pos-chunks
            at8_t = ppa.tile([128, 8, 512], FP8, tag="at8")
            hst_t = ppa.tile([128, 8, ROWS], BF16, tag="hst")      # hsT history
            hst8_t = ppa.tile([128, 8, ROWS], FP8, tag="hst8")     # fp8 (x H_SCALE)
            h0t_t = ppa.tile([128, 8, B], BF16, tag="h0t")
            h08_t = ppa.tile([128, 8, B], FP8, tag="h08")
            cc_t = ppa.tile([128, 512], F32, tag="cc")             # [tg | c]
            i128_t = ppa.tile([128, 128], BF16, tag="i128")
            m32_t = ppa.tile([32, 512], BF16, tag="m32")
            se_t = ppa.tile([128, NBLK, VCH], F32, tag="SE")
            tga_t = [ppa.tile([128, 8, B], F32, tag=f"tga{i}", name=f"tga{i}")
                     for i in range(2)]                            # tgt-score accum
            nc.sync.dma_start(i128_t[:], d_i128[:])
            nc.sync.dma_start(m32_t[:], d_m32[:])
            nc.vector.memset(se_t[:], 1.0)   # ln(1)=0 for padded rows
            nc.vector.memset(tga_t[0][:], 0.0)
            # recurrence weights prefetched on the scalar HWDGE queue
            wh_t = ppb.tile([128, 8, 4096], BF16, tag="wh")
            wx_t = ppb.tile([128, 4, 4096], BF16, tag="wx")
            xt_t = ppb.tile([128, 4, ROWS], BF16, tag="xt")
            nc.scalar.dma_start(
                wh_t[:], d_wh[:].rearrange("(c k) m -> k c m", k=128))
            nc.scalar.dma_start(
                wx_t[:], d_wx[:].rearrange("(c k) m -> k c m", k=128))
            nc.scalar.dma_start(
                xt_t[:], d_xt[:].rearrange("(c k) m -> k c m", k=128))
            if has_b:
                bvec_t = ppa.tile([1, 4096], BF16, tag="bvec")
                ones_t = ppa.tile([1, 128], BF16, tag="ones")
                nc.sync.dma_start(bvec_t[:], d_bvec[:])
                nc.vector.memset(ones_t[:], 1.0)

            # ================= P1: feature projection -> A2T, h0, c0 ==========
            with (
                tc.tile_pool(name="p12", bufs=1) as p12,
                tc.tile_pool(name="psa", bufs=2, space="PSUM") as psa,
            ):
                f2t_t = p12.tile([128, 11, 512], BF16, tag="f2t")
                wproj_t = p12.tile([128, 11, 1024], BF16, tag="wproj")
                nc.sync.dma_start(
                    f2t_t[:], d_f2t[:].rearrange("(c k) m -> k c m", k=128))
                nc.sync.dma_start(
                    wproj_t[:], d_wproj[:].rearrange("(c k) m -> k c m", k=128))
                h0f_t = p12.tile([128, 8, B], F32, tag="h0f")
                for hc in range(8):
                    ps = psa.tile([128, 512], F32, tag="pp", name=f"pp1_{hc}")
                    for kk in range(11):
                        nc.tensor.matmul(
                            ps[:], wproj_t[:, kk, 128 * hc:128 * (hc + 1)],
                            f2t_t[:, kk, :], start=(kk == 0), stop=(kk == 10))
                    nc.vector.tensor_copy(at_t[:, POS[hc], :], ps[:])
                    nc.scalar.activation(at8_t[:, POS[hc], :], ps[:], AF.Copy)
                    nc.vector.reduce_sum(
                        h0f_t[:, POS[hc], :],
                        at_t[:, POS[hc], :].rearrange("k (n p) -> k n p", p=P16),
                        axis=AX.X)

                # h0 = mean over p (h0f is the sum); h08 = h0 * 16 = h0f
                nc.vector.tensor_scalar(h0t_t[:], h0f_t[:],
                                        1.0 / P16, None, op0=ALU.mult)
                nc.vector.tensor_copy(h08_t[:], h0f_t[:])
                c0p = psa.tile([64, 512], BF16, tag="c0p")
                for kh in range(8):
                    eta, j = kh // 4, kh % 4
                    nc.tensor.transpose(
                        c0p[32 * eta:32 * (eta + 1), 128 * j:128 * (j + 1)],
                        h0t_t[:, POS[kh], :], i128_t[:, 0:128],
                        tile_position=(0, 32 * eta))
                nc.vector.tensor_copy(cc_t[64:128, :], c0p[:])

            # bp pool opens after P1 frees wproj/f2t
            with tc.tile_pool(name="ppc", bufs=1) as ppc:
                bp_t = [ppc.tile([128, 4096], BF16, tag=f"bp{c}", name=f"bp{c}")
                        for c in range(4)]

                # ================= P2: B = A2 @ Wattn ==========
                with (
                    tc.tile_pool(name="p2w", bufs=2) as p2w,
                    tc.tile_pool(name="psb", bufs=2, space="PSUM") as psb,
                ):
                    for v in range(8):
                        wat_t = p2w.tile([128, 8, 512], BF16, tag="wat")
                        nc.sync.dma_start(
                            wat_t[:],
                            d_wattn[:, 512 * v:512 * (v + 1)]
                            .rearrange("(c k) m -> k c m", k=128))
                        for c in range(4):
                            ps = psb.tile([128, 512], F32, tag="pp",
                                          name=f"pp2_{v}_{c}")
                            for kp in range(8):
                                nc.tensor.matmul(
                                    ps[:], at_t[:, kp, 128 * c:128 * (c + 1)],
                                    wat_t[:, kp, :], start=(kp == 0),
                                    stop=(kp == 7))
                            nc.vector.tensor_copy(
                                bp_t[c][:, 512 * v:512 * (v + 1)], ps[:])

                # ================= P3: recurrence + interleaved vocab ==========
                with (
                    tc.tile_pool(name="ps3", bufs=2, space="PSUM") as ps3,
                    tc.tile_pool(name="psS", bufs=1, space="PSUM") as psSp,
                    tc.tile_pool(name="psT", bufs=1, space="PSUM") as psTp,
                    tc.tile_pool(name="psV", bufs=2, space="PSUM") as psVp,
                    tc.tile_pool(name="wk3", bufs=2) as wk3,
                    tc.tile_pool(name="wk3g", bufs=1) as wk3g,
                    tc.tile_pool(name="wk3h", bufs=2) as wk3h,
                    tc.tile_pool(name="wkv", bufs=2) as wkv,
                    tc.tile_pool(name="wkt", bufs=2) as wkt,
                ):
                    def ht_lhs(t, pos):
                        if t == 0:
                            return hst_t[:, pos, 0:B]  # unused placeholder
                        return hst_t[:, pos, B * (t - 1):B * t]

                    def emit_x(t2):
                        pA = ps3.tile([128, 512], F32, tag="pA", name=f"pA{t2}")
                        pB = ps3.tile([128, 512], F32, tag="pB", name=f"pB{t2}")
                        for c2 in range(4):
                            for u2, (g2, e2) in enumerate(units):
                                ps2, j2 = (pA, u2) if u2 < 4 else (pB, u2 - 4)
                                lo2 = 1024 * g2 + 512 * e2
                                sl2 = slice(32 * j2, 32 * (j2 + 1))
                                nc.tensor.matmul(
                                    ps2[sl2, :], xt_t[:, c2, B * t2:B * (t2 + 1)],
                                    wx_t[:, c2, lo2:lo2 + 512],
                                    start=(c2 == 0), stop=False,
                                    tile_position=(0, 32 * j2),
                                    skip_group_check=True)
                        return pA, pB

                    # vocab work items (block, vc), scheduled per step
                    vq = {t: [] for t in range(S)}
                    for b in range(7):
                        t0v = 4 * b + 4
                        for i in range(VCH):
                            if b < 6:
                                tv = t0v + i // 5
                            else:
                                tv = t0v + min(i // 7, 2)
                            vq[tv].append((b, i))
                    vtail = [(7, i) for i in range(VCH)]

                    def emit_vocab_mms(b, vc, qi):
                        nr = 96 if b == 7 else 128
                        wv_t = wkv.tile([128, 8, 512], FP8, tag="wv",
                                        name=f"wv{b}_{vc}")
                        eng = nc.sync if qi % 2 == 0 else nc.scalar
                        eng.dma_start(
                            wv_t[:, :, 0:VCOL],
                            d_wvoc8[:, VCOL * vc:VCOL * (vc + 1)]
                            .rearrange("(c k) m -> k c m", k=128))
                        pv = psVp.tile([128, 512], F32, tag="pv",
                                       name=f"pv{b}_{vc}")
                        for kp in range(4):
                            nc.tensor.matmul(
                                pv[0:nr, 0:VCOL],
                                hst8_t[:, 2 * kp:2 * kp + 2, 128 * b:128 * b + nr],
                                wv_t[:, 2 * kp:2 * kp + 2, 0:VCOL],
                                start=(kp == 0), stop=(kp == 3), perf_mode=DR)
                        return pv, nr

                    def emit_vocab_exp(pv, b, vc, nr):
                        scr = wk3h.tile([128, 512], BF16, tag="scr",
                                        name=f"scr{b}_{vc}")
                        nc.scalar.activation(
                            scr[0:nr, 0:VCOL], pv[0:nr, 0:VCOL], AF.Exp,
                            scale=float(1.0 / (H_SCALE * W_SCALE)),
                            accum_out=se_t[0:nr, b, vc:vc + 1])

                    qi = 0
                    ps_cur = emit_x(0)
                    for t in range(S):
                        with tc.high_priority(offset=PRIO):
                            # ---- attention scores (h from step t-1, fp8 DR)
                            psS = psSp.tile([32, 512], F32, tag="pS",
                                            name=f"pS{t}")
                            nc.tensor.matmul(psS[:], i128_t[0:32, 0:32],
                                             m32_t[:], start=True, stop=False)
                            h8 = (h08_t if t == 0 else
                                  hst8_t[:, :, B * (t - 1):B * t])
                            for kp in range(4):
                                nc.tensor.matmul(
                                    psS[:], h8[:, 2 * kp:2 * kp + 2, :],
                                    at8_t[:, 2 * kp:2 * kp + 2, :],
                                    start=False, stop=(kp == 3), perf_mode=DR)

                        # ---- gate GEMM h-part (k-outer, normal priority)
                        psA, psB = ps_cur
                        for pos in range(8):
                            hp = (h0t_t[:, pos, :] if t == 0
                                  else ht_lhs(t, pos))
                            for u, (g, eta) in enumerate(units):
                                ps, j = (psA, u) if u < 4 else (psB, u - 4)
                                lo = 1024 * g + 512 * eta
                                nc.tensor.matmul(
                                    ps[32 * j:32 * (j + 1), :], hp,
                                    wh_t[:, pos, lo:lo + 512],
                                    start=False, stop=False,
                                    tile_position=(0, 32 * j),
                                    skip_group_check=True)

                        with tc.high_priority(offset=PRIO):
                            # ---- softmax
                            e_t = wk3.tile([32, 512], BF16, tag="e",
                                           name=f"e{t}")
                            se_sm = wk3.tile([32, 1], F32, tag="sesm",
                                             name=f"sesm{t}")
                            nc.scalar.activation(
                                e_t[:], psS[:], AF.Exp,
                                scale=float(1.0 / (H_SCALE * np.sqrt(H))),
                                accum_out=se_sm[:, 0:1])
                            re_t = wk3.tile([32, 1], F32, tag="re",
                                            name=f"re{t}")
                            nc.vector.reciprocal(re_t[:], se_sm[:])
                            w_t = wk3.tile([32, 512], BF16, tag="w",
                                           name=f"w{t}")
                            nc.vector.tensor_scalar(w_t[:], e_t[:],
                                                    re_t[:, 0:1], None,
                                                    op0=ALU.mult)
                            # ---- transpose w -> wT chunks
                            pT = psTp.tile([128, 4, 2, 32], BF16, tag="pT",
                                           name=f"pTw{t}")
                            for j in range(4):
                                nc.tensor.transpose(
                                    pT[:, j, 0, :],
                                    w_t[:, 128 * j:128 * (j + 1)],
                                    i128_t[0:32, 0:32])
                            wt_t = wk3.tile([128, 4, 32], BF16, tag="wt",
                                            name=f"wt{t}")
                            nc.vector.tensor_copy(wt_t[:], pT[:, :, 0, :])

                            # ---- gate GEMM attn-part (c-outer)
                            for c in range(4):
                                for u, (g, eta) in enumerate(units):
                                    ps, j = (psA, u) if u < 4 else (psB, u - 4)
                                    lo = 1024 * g + 512 * eta
                                    nc.tensor.matmul(
                                        ps[32 * j:32 * (j + 1), :],
                                        wt_t[:, c, :],
                                        bp_t[c][:, lo:lo + 512],
                                        start=False,
                                        stop=(c == 3 and not has_b),
                                        tile_position=(0, 32 * j),
                                        skip_group_check=True)
                            if has_b:
                                for u, (g, eta) in enumerate(units):
                                    ps, j = (psA, u) if u < 4 else (psB, u - 4)
                                    lo = 1024 * g + 512 * eta
                                    nc.tensor.matmul(
                                        ps[32 * j:32 * (j + 1), :],
                                        ones_t[0:1, 0:32],
                                        bvec_t[0:1, lo:lo + 512],
                                        start=False, stop=True,
                                        tile_position=(0, 32 * j),
                                        skip_group_check=True)

                        # ---- x-part of next step (pipelined)
                        if t + 1 < S:
                            ps_next = emit_x(t + 1)

                        # ---- vocab matmuls (fill PE idle in act window)
                        vitems = []
                        for (b, vc) in vq[t]:
                            pv, nr = emit_vocab_mms(b, vc, qi)
                            qi += 1
                            vitems.append((pv, b, vc, nr))

                        # ---- target-score partial on GpSimd (idle engine)
                        if t > 0:
                            wtg_t = wkt.tile([128, 8, B], BF16, tag="wtg",
                                             name=f"wtg{t}")
                            nc.scalar.dma_start(
                                wtg_t[:],
                                d_wtgt[:, B * (t - 1):B * t]
                                .rearrange("(c k) m -> k c m", k=128))
                            prod_t = wkt.tile([128, 8, B], F32, tag="prod",
                                              name=f"prod{t}")
                            nc.gpsimd.tensor_tensor(
                                prod_t[:], hst_t[:, :, B * (t - 1):B * t],
                                wtg_t[:], op=ALU.mult)
                            nc.gpsimd.tensor_tensor(
                                tga_t[t % 2][:], tga_t[(t + 1) % 2][:],
                                prod_t[:], op=ALU.add)

                        with tc.high_priority(offset=PRIO):
                            # ---- gates
                            nc.scalar.activation(psA[:], psA[:], AF.Tanh,
                                                 scale=0.5)
                            nc.scalar.activation(cc_t[0:64, :], psB[64:128, :],
                                                 AF.Tanh)
                            to_t = wk3g.tile([64, 512], BF16, tag="to")
                            nc.scalar.activation(to_t[:], psB[0:64, :],
                                                 AF.Tanh, scale=0.5)
                            sfsi_t = wk3g.tile([128, 512], F32, tag="sfsi")
                            nc.vector.tensor_scalar(sfsi_t[:], psA[:], 0.5, 0.5,
                                                    op0=ALU.mult, op1=ALU.add)
                            v_t = wk3g.tile([64, 512], F32, tag="v")
                            nc.vector.tensor_tensor(v_t[:], sfsi_t[0:64, :],
                                                    cc_t[0:64, :], op=ALU.mult)
                            u_t = wk3g.tile([64, 512], F32, tag="u")
                            nc.vector.tensor_tensor(u_t[:], sfsi_t[64:128, :],
                                                    cc_t[64:128, :],
                                                    op=ALU.mult)
                            nc.vector.tensor_tensor(cc_t[64:128, :], u_t[:],
                                                    v_t[:], op=ALU.add)
                            tc_t = wk3h.tile([64, 512], BF16, tag="tc")
                            nc.scalar.activation(tc_t[:], cc_t[64:128, :],
                                                 AF.Tanh)
                            so_t = wk3g.tile([64, 512], BF16, tag="so")
                            nc.gpsimd.tensor_scalar(so_t[:], to_t[:], 0.5, 0.5,
                                                    op0=ALU.mult, op1=ALU.add)
                            hf_t = wk3.tile([64, 512], BF16, tag="hf",
                                            name=f"hf{t}")
                            nc.vector.tensor_tensor(hf_t[:], so_t[:], tc_t[:],
                                                    op=ALU.mult)

                            # ---- transpose h -> hT (4x paired [64,128])
                            pH = psTp.tile([128, 4, 2, 32], BF16, tag="pT",
                                           name=f"pTh{t}")
                            for m in range(4):
                                nc.tensor.transpose(
                                    pH[:, m].rearrange("k a n -> k (a n)"),
                                    hf_t[:, 128 * m:128 * (m + 1)],
                                    i128_t[0:64, 0:64])
                            pHv = pH[:].rearrange("k m a n -> k (m a) n")
                            nc.vector.tensor_copy(
                                hst_t[:, :, B * t:B * (t + 1)], pHv)
                            nc.vector.tensor_scalar(
                                hst8_t[:, :, B * t:B * (t + 1)], pHv,
                                H_SCALE, None, op0=ALU.mult)

                        # keep-warm anchor for the first (vocab-less) steps
                        if t < 4:
                            pD = psTp.tile([128, 4, 2, 32], BF16, tag="pT",
                                           name=f"pdum{t}")
                            nc.tensor.transpose(
                                pD[0:64, 0].rearrange("k a n -> k (a n)"),
                                to_t[:, 0:64], i128_t[0:64, 0:64])

                        # ---- vocab exps (low priority, fill ACT idle)
                        for (pv, b, vc, nr) in vitems:
                            emit_vocab_exp(pv, b, vc, nr)

                        if t + 1 < S:
                            ps_cur = ps_next

                    # last step's target partial
                    wtg_t = wkt.tile([128, 8, B], BF16, tag="wtg",
                                     name="wtgS")
                    nc.scalar.dma_start(
                        wtg_t[:],
                        d_wtgt[:, B * (S - 1):B * S]
                        .rearrange("(c k) m -> k c m", k=128))
                    prod_t = wkt.tile([128, 8, B], F32, tag="prod",
                                      name="prodS")
                    nc.gpsimd.tensor_tensor(
                        prod_t[:], hst_t[:, :, B * (S - 1):B * S],
                        wtg_t[:], op=ALU.mult)
                    nc.gpsimd.tensor_tensor(
                        tga_t[S % 2][:], tga_t[(S + 1) % 2][:],
                        prod_t[:], op=ALU.add)

                    # tail vocab block (rows of steps 28-30)
                    for (b, vc) in vtail:
                        pv, nr = emit_vocab_mms(b, vc, qi)
                        qi += 1
                        emit_vocab_exp(pv, b, vc, nr)

            # ================= P4: reduce to loss ==========
            with (
                tc.tile_pool(name="p4", bufs=1) as p4,
            ):
                tacc = p4.tile([128, 1], F32, tag="tacc")
                nc.vector.reduce_sum(
                    tacc[:], tga_t[S % 2][:].rearrange("k a n -> k (a n)"),
                    axis=AX.X)
                tgt_r = p4.tile([128, 1], F32, tag="tgtr")
                nc.gpsimd.partition_all_reduce(tgt_r[:], tacc[:], channels=128,
                                               reduce_op=bass_isa.ReduceOp.add)

                ses_t = p4.tile([128, NBLK], F32, tag="ses")
                nc.vector.reduce_sum(ses_t[:], se_t[:], axis=AX.X)
                l_t = p4.tile([128, NBLK], F32, tag="lt")
                nc.scalar.activation(l_t[:], ses_t[:], AF.Ln)
                maskm_t = p4.tile([128, NBLK], F32, tag="maskm")
                nc.sync.dma_start(maskm_t[:], d_maskm[:])
                lm_t = p4.tile([128, NBLK], F32, tag="lm")
                nc.vector.tensor_tensor(lm_t[:], l_t[:], maskm_t[:], op=ALU.mult)
                lr_t = p4.tile([128, 1], F32, tag="lr")
                nc.vector.reduce_sum(lr_t[:], lm_t[:], axis=AX.X)
                lse_r = p4.tile([128, 1], F32, tag="lser")
                nc.gpsimd.partition_all_reduce(lse_r[:], lr_t[:], channels=128,
                                               reduce_op=bass_isa.ReduceOp.add)

                nll_t = p4.tile([1, 1], F32, tag="nll")
                nc.vector.tensor_tensor(nll_t[:], lse_r[0:1, :], tgt_r[0:1, :],
                                        op=ALU.subtract)
                if has_bvocab:
                    btgt_t = p4.tile([1, ROWS], F32, tag="btgt")
                    nc.sync.dma_start(btgt_t[:], d_btgt[:])
                    bts_t = p4.tile([1, 1], F32, tag="bts")
                    nc.vector.reduce_sum(bts_t[:], btgt_t[:], axis=AX.X)
                    nc.vector.tensor_tensor(nll_t[:], nll_t[:], bts_t[:],
                                            op=ALU.subtract)
                loss_t = p4.tile([1, 1], F32, tag="loss")
                nc.vector.tensor_scalar(loss_t[:], nll_t[:], 1.0 / N, None,
                                        op0=ALU.mult)
                nc.sync.dma_start(d_loss[:], loss_t[:])

    nc.finalize()
    return nc


def _perm_rows(a):
    """Permute the 8x128 h-dim row chunks of a (1024, X) array to pos order."""
    return a.reshape(8, 128, -1)[CHUNK_ORDER].reshape(1024, a.shape[1])


def kernel(features, captions, W_proj, b_proj, W_embed, Wx, Wh, Wattn, b,
           W_vocab, b_vocab):
    global last_exec_ns
    from concourse.bass_utils import run_bass_kernel_spmd

    features = np.asarray(features)
    captions = np.asarray(captions)
    W_proj = np.asarray(W_proj, np.float32)
    b_proj = np.asarray(b_proj, np.float32)
    W_embed = np.asarray(W_embed, np.float32)
    Wx = np.asarray(Wx, np.float32)
    Wh = np.asarray(Wh, np.float32)
    Wattn = np.asarray(Wattn, np.float32)
    b = np.asarray(b, np.float32)
    W_vocab = np.asarray(W_vocab, np.float32)
    b_vocab = np.asarray(b_vocab, np.float32)

    has_b = bool(np.any(b))
    has_bvocab = bool(np.any(b_vocab))

    key = (has_b, has_bvocab)
    if key not in _cache:
        _cache[key] = _build(has_b, has_bvocab)
    nc = _cache[key]

    cap_in = np.asarray(captions[:, :-1], np.int64)   # (N, S)
    cap_out = np.asarray(captions[:, 1:], np.int64)
    mask = (cap_out != 0).astype(np.float32)          # (N, S)
    x = W_embed[cap_in].astype(np.float32)            # (N, S, W_DIM)

    wproj_h = np.zeros((1408, 1024), np.float32)
    wproj_h[:D_IMG] = W_proj
    wproj_h[D_IMG] = b_proj
    wproj_h = wproj_h.astype(BF)
    wh_h = _perm_rows(Wh).astype(BF)
    wx_h = Wx.astype(BF)
    wattn_h = _perm_rows(Wattn).astype(BF)
    wvoc8_h = (_perm_rows(W_vocab) * W_SCALE).astype(F8)
    i128_h = np.eye(128, dtype=BF)
    col_n = np.arange(B * P16) // P16
    m32_h = np.where(col_n[None, :] == np.arange(B)[:, None], 0.0, NEG
                     ).astype(BF)

    feat = features.reshape(N, D_IMG, P16).astype(np.float32)

    in_maps = []
    for ci in range(NC):
        sl = slice(ci * B, (ci + 1) * B)
        f2t = np.zeros((1408, 512), np.float32)
        f2t[:D_IMG] = feat[sl].transpose(1, 0, 2).reshape(D_IMG, B * P16)
        f2t[D_IMG] = 1.0
        xt = x[sl].transpose(2, 1, 0).reshape(W_DIM, ROWS)  # col = 32*t + n
        tgt = cap_out[sl].T.reshape(ROWS)                   # r = 32*t + n
        mk = mask[sl].T.reshape(ROWS)
        wtgt = _perm_rows(W_vocab[:, tgt] * mk[None, :]).astype(BF)
        mkp = np.zeros(128 * NBLK, np.float32)
        mkp[:ROWS] = mk
        maskm = mkp.reshape(NBLK, 128).T.copy()             # [row, blk]
        m = {
            "f2t": f2t.astype(BF),
            "wproj": wproj_h,
            "wattn": wattn_h,
            "wh": wh_h,
            "wx": wx_h,
            "xt": xt.astype(BF),
            "wvoc8": wvoc8_h,
            "wtgt": wtgt,
            "maskm": maskm,
            "i128": i128_h,
            "m32": m32_h,
        }
        if has_b:
            m["bvec"] = b.reshape(1, 4096).astype(BF)
        if has_bvocab:
            m["bvoc"] = b_vocab.reshape(1, V).astype(np.float32)
            m["btgt"] = (b_vocab[tgt] * mk).reshape(1, ROWS).astype(np.float32)
        in_maps.append(m)

    trace = bool(int(os.environ.get("BASS_KPROF", "0")))
    if trace:
        import sys, types
        try:
            import antenv.axon_hooks  # noqa
        except ImportError:
            import trn_agent_boot.trn_boot as _tb
            _hook = _tb._ntff_profile_via_ctypes("/opt/axon/libaxon_pjrt.so")
            _mod = types.ModuleType("antenv.axon_hooks")
            _mod.get_axon_ntff_profile_hook = lambda: _hook
            import antenv
            sys.modules["antenv.axon_hooks"] = _mod
            antenv.axon_hooks = _mod

    if os.environ.get("BASS_SIM"):
        from concourse.bass_interp import CoreSim
        sim = CoreSim(nc)
        for k2, v2 in in_maps[0].items():
            sim.tensor(k2)[:] = v2
        sim.simulate()
        print("SIM core0 partial loss:", np.asarray(sim.tensor("loss"))[0, 0],
              flush=True)
        return np.asarray(np.float32(np.asarray(sim.tensor("loss"))[0, 0] * NC))

    res = run_bass_kernel_spmd(nc, in_maps, core_ids=list(range(NC)),
                               trace=trace)
    last_exec_ns = res.exec_time_ns
    total = np.float32(0.0)
    for ci in range(NC):
        total += res.results[ci]["loss"][0, 0]
    out = np.asarray(total, np.float32)
    return out
